# revision 58
# baseline (speedup 1.0000x reference)
"""Trainium2 Bass kernel for nn_Block_523986010339 (PVT-style transformer block).

Sharding: data-parallel over batch B=8 -> one batch element per NeuronCore.
Per-core design (v2):
  - x loaded with partition p = token//128: 32KB contiguous DMA lines; the
    image arrives transposed, conv taps are transposed host-side
  - residual stream token-major fp32 [128p, 128q, 64c]; channel-major
    operands bf16 [c, n], n = 128*q + p
  - channel-major tiles carry doubled rows: partitions 64:128 = rows 0:64
    shifted one column; consumers use them for 2-tap K=128 matmuls and for
    row-tiled K=64 matmul pairs (tile_position via base_partition)
  - LN stats finalized per 16-column slice so PE transposes start ~4us in
    (keeps the HAM clock-gate warm from the start)
  - attention: S^T [256kv, n] via row-tiled K=64 pairs into bf16 PSUM
    (1024-wide), exp without max-subtraction, proj folded into V host-side
    (wkv V-half pre-multiplied by proj^T) so O = E^T @ Vp is token-major
    directly with a fused denominator column
  - MLP: fc1 + 3x3 depthwise fused into 9 taps; 6 K=128 2-tap matmuls plus
    3 row-tiled K=64 pair-slots per 512-chunk; fc2 output written to both
    halves of a doubled o2cm so epilogue transposes handle 2 image rows each
  - a2 build rides behind attention chunks; MLP chunks and epilogue are
    emitted as soon as their a2g/o2cm spans are complete
"""

import functools
import json

import numpy as np
import ml_dtypes

import concourse.bass as bass
import concourse.mybir as mybir
import concourse.tile as tile
from concourse.bass_utils import run_bass_kernel_spmd
from concourse.masks import make_identity

F32 = mybir.dt.float32
BF16 = mybir.dt.bfloat16
BF = ml_dtypes.bfloat16

B, N, C, H, W = 8, 16384, 64, 128, 128
SR, HID, NR = 8, 256, 256
P, T = 128, 128
RP = W + 2          # guarded row pitch
PAD = RP + 1        # head/tail pad so all tap offsets stay in-bounds
NG = PAD + RP * (H + 2) + PAD
AX = mybir.AxisListType
OP = mybir.AluOpType
AF = mybir.ActivationFunctionType


def _split_excess_waits(nc, max_waits=1):
    """walrus in this container rejects >1 sync wait per instruction; move
    excess waits onto injected Drain instructions just before the owner."""
    d = json.loads(mybir.module_to_json_string(nc.m))
    n_split = [0]

    def fix(insts):
        out = []
        for inst in insts:
            si = inst.get("sync_info") or {}
            waits = si.get("on_wait") or []
            if len(waits) > max_waits:
                extra = waits[:-max_waits]
                for i in range(0, len(extra), max_waits):
                    n_split[0] += 1
                    out.append({
                        "name": f"WSPLIT-{n_split[0]}",
                        "opcode": "NoOp",
                        "engine": inst["engine"],
                        "ins": [],
                        "outs": [],
                        "is_reset_sema": False,
                        "sync_info": {"on_update": [],
                                      "on_wait": extra[i:i + max_waits]},
                    })
                si["on_wait"] = waits[-max_waits:]
                inst["sync_info"] = si
            out.append(inst)
        return out

    for f in d.get("functions", []):
        for bb in f.get("blocks", []):
            bb["instructions"] = fix(bb["instructions"])
    nc.m = mybir.module_from_json_string(json.dumps(d))


def _ln_stats(nc, sc, big, x_tm, epst, nt):
    """Token-major LN stats: returns (g, mg) tiles [128, nt] fp32 given
    x_tm [128, nt, 64] fp32."""
    sq_scr = big.tile([P, nt * C], BF16, tag="scr2", name="sq")
    xsq_view = sq_scr.rearrange("p (t c) -> p t c", c=C)
    nc.scalar.square(out=sq_scr, in_=x_tm.rearrange("p t c -> p (t c)"))
    s1 = sc.tile([P, nt], F32, tag=f"s1_{nt}")
    s2 = sc.tile([P, nt], F32, tag=f"s2_{nt}")
    nc.vector.tensor_reduce(out=s1, in_=x_tm, axis=AX.X, op=OP.add)
    nc.vector.tensor_reduce(out=s2, in_=xsq_view, axis=AX.X, op=OP.add)
    g = sc.tile([P, nt], F32, tag=f"gx_{nt}")
    mg = sc.tile([P, nt], F32, tag=f"mgx_{nt}")
    _ln_finalize(nc, sc, s1, s2, epst, nt, g, mg)
    return g, mg


def _ln_finalize(nc, sc, s1, s2, epst, nt, g_out, mg_out, tg=""):
    """Finalize LN stats s1/s2 [128, nt] into g_out/mg_out (slices ok)."""
    mean = sc.tile([P, nt], F32, tag=f"mean_{nt}{tg}")
    var = sc.tile([P, nt], F32, tag=f"var_{nt}{tg}")
    nc.vector.tensor_scalar_mul(out=mean, in0=s1, scalar1=1.0 / C)
    nc.vector.tensor_scalar_mul(out=var, in0=s2, scalar1=1.0 / C)
    mm = sc.tile([P, nt], F32, tag=f"mm_{nt}{tg}")
    nc.vector.tensor_tensor(out=mm, in0=mean, in1=mean, op=OP.mult)
    nc.vector.tensor_tensor(out=var, in0=var, in1=mm, op=OP.subtract)
    sd = sc.tile([P, nt], F32, tag=f"sd_{nt}{tg}")
    nc.scalar.activation(out=sd, in_=var, func=AF.Sqrt, bias=epst, scale=1.0)
    nc.vector.reciprocal(out=g_out, in_=sd)
    nc.vector.tensor_tensor(out=mg_out, in0=mean, in1=g_out, op=OP.mult)


def _build_nc():
    nc = bass.Bass("TRN2")
    x_d = nc.dram_tensor("x", [N, C], F32, kind="ExternalInput")
    out_d = nc.dram_tensor("out", [N, C], F32, kind="ExternalOutput")
    wq_d = nc.dram_tensor("wq", [C, C], BF16, kind="ExternalInput")
    bq_d = nc.dram_tensor("bq", [C, 1], F32, kind="ExternalInput")
    wsr_d = nc.dram_tensor("wsr", [128, 32, C], BF16, kind="ExternalInput")
    bsr_d = nc.dram_tensor("bsr", [C, 1], F32, kind="ExternalInput")
    wkv_d = nc.dram_tensor("wkv", [C, 2 * C], BF16, kind="ExternalInput")
    bkv_d = nc.dram_tensor("bkv", [2 * C, 1], F32, kind="ExternalInput")
    wmp_d = nc.dram_tensor("wmp", [128, 6, 128], BF16, kind="ExternalInput")
    wms_d = nc.dram_tensor("wms", [128, 3, 128], BF16, kind="ExternalInput")
    bg_d = nc.dram_tensor("bg", [128, 2], F32, kind="ExternalInput")
    wf2_d = nc.dram_tensor("wf2", [128, 2, C], BF16, kind="ExternalInput")
    bf2_d = nc.dram_tensor("bf2", [C, 1], F32, kind="ExternalInput")

    with tile.TileContext(nc) as tc:
        with (
            tc.tile_pool(name="consts", bufs=1) as consts,
            tc.tile_pool(name="big", bufs=1) as big,
            tc.tile_pool(name="sc", bufs=2) as sc,
            tc.tile_pool(name="ch", bufs=3) as ch,
            tc.tile_pool(name="psT", bufs=2, space="PSUM") as psT,
            tc.tile_pool(name="psS", bufs=4, space="PSUM") as psS,
            tc.tile_pool(name="psP", bufs=2, space="PSUM") as psP,
        ):
            ident = consts.tile([128, 128], BF16)
            make_identity(nc, ident)
            wq = consts.tile([C, C], BF16)
            nc.gpsimd.dma_start(out=wq, in_=wq_d[:, :])
            wsr = consts.tile([128, 32, C], BF16)
            nc.gpsimd.dma_start(out=wsr, in_=wsr_d[:, :, :])
            wkv = consts.tile([C, 2 * C], BF16)
            nc.gpsimd.dma_start(out=wkv, in_=wkv_d[:, :])
            wmp = consts.tile([128, 6, 128], BF16)
            nc.gpsimd.dma_start(out=wmp, in_=wmp_d[:, :, :])
            wms = consts.tile([128, 3, 128], BF16)
            nc.gpsimd.dma_start(out=wms, in_=wms_d[:, :, :])
            wf2 = consts.tile([128, 2, C], BF16)
            nc.gpsimd.dma_start(out=wf2, in_=wf2_d[:, :, :])
            bq = consts.tile([C, 1], F32)
            nc.gpsimd.dma_start(out=bq, in_=bq_d[:, :])
            bsr = consts.tile([C, 1], F32)
            nc.gpsimd.dma_start(out=bsr, in_=bsr_d[:, :])
            bkv = consts.tile([2 * C, 1], F32)
            nc.gpsimd.dma_start(out=bkv, in_=bkv_d[:, :])
            bg = consts.tile([128, 2], F32)
            nc.gpsimd.dma_start(out=bg, in_=bg_d[:, :])
            bf2 = consts.tile([C, 1], F32)
            nc.gpsimd.dma_start(out=bf2, in_=bf2_d[:, :])
            epst = consts.tile([P, 1], F32)
            nc.vector.memset(epst, 1e-5)

            # PE warmup: ~3.2us of back-to-back matmuls so the HAM clock
            # gate reaches 8/8 before the first real transposes arrive
            for wd in range(30):
                pw = psT.tile([128, 128], F32, tag="tp", name="pw")
                nc.tensor.matmul(out=pw, lhsT=ident, rhs=ident,
                                 start=True, stop=True)

            # ---- load x (contiguous lines), LN1 per-slice, build a1cm ----
            x_tm = big.tile([P, T, C], F32, tag="xr")
            x_v = x_d.rearrange("(p t) c -> p t c", t=T)
            s1 = sc.tile([P, T], F32, tag="s1")
            a1tm = big.tile([P, T, C], BF16, tag="scr2")
            a1cm = big.tile([128, N], BF16, tag="acm")
            a1cm_v = a1cm[0:C, :].rearrange("c (j a b n) -> c j a b n", a=4, b=2, n=128)
            a1tm_v = a1tm.rearrange("p t c -> p (t c)")

            def tree_sum(dst, src3, nt, wid, f32, eng=None):
                """dst[:, 0:nt] = sum over last axis of src3 [P, nt, wid]."""
                eng = eng or nc.vector
                t1 = sc.tile([P, nt, wid // 2], F32 if f32 else BF16,
                             tag=f"tr1_{nt}_{wid}{f32}")
                eng.tensor_tensor(out=t1, in0=src3[:, :, 0:wid // 2],
                                  in1=src3[:, :, wid // 2:wid], op=OP.add)
                t2 = sc.tile([P, nt, wid // 4], F32, tag=f"tr2_{nt}_{wid}{f32}")
                eng.tensor_tensor(out=t2, in0=t1[:, :, 0:wid // 4],
                                  in1=t1[:, :, wid // 4:wid // 2], op=OP.add)
                if eng is nc.vector:
                    eng.tensor_reduce(out=dst, in_=t2, axis=AX.X, op=OP.add)
                    return
                # gpsimd cannot reduce along X: finish with pairwise adds
                w = wid // 4
                cur = t2
                while w > 2:
                    nxt = sc.tile([P, nt, w // 2], F32,
                                  tag=f"tr_{nt}_{w}{f32}")
                    eng.tensor_tensor(out=nxt, in0=cur[:, :, 0:w // 2],
                                      in1=cur[:, :, w // 2:w], op=OP.add)
                    cur, w = nxt, w // 2
                eng.tensor_tensor(out=dst, in0=cur[:, :, 0],
                                  in1=cur[:, :, 1], op=OP.add)

            # first slice split in half: the opening transposes wait on a
            # full DMA->square->sum->finalize->scale chain, so a smaller
            # first slice halves the pipeline-fill latency
            spans = [(0, 8), (8, 16)] + [(16 * q8, 16 * (q8 + 1))
                                         for q8 in range(1, 8)]
            for lo, hi in spans:
                sl = slice(lo, hi)
                nt = hi - lo
                nc.sync.dma_start(out=x_tm[:, sl, :], in_=x_v[:, sl, :])
                sq = sc.tile([P, nt, C], BF16, tag=f"sq16_{nt}")
                nc.scalar.square(out=sq, in_=x_tm[:, sl, :])
                # per-slice ring tiles: slicing one shared stats tile would
                # chain slices through whole-tile write-after-read hazards
                s1r = sc.tile([P, nt], F32, tag=f"s1r{nt}")
                s2r = sc.tile([P, nt], F32, tag=f"s2r{nt}")
                g1r = sc.tile([P, nt], F32, tag=f"g1r{nt}")
                mg1r = sc.tile([P, nt], F32, tag=f"mg1r{nt}")
                tree_sum(s1r, x_tm[:, sl, :], nt, C, True)
                tree_sum(s2r, sq, nt, C, False)
                nc.gpsimd.tensor_copy(out=s1[:, sl], in_=s1r)
                _ln_finalize(nc, sc, s1r, s2r, epst, nt, g1r, mg1r)
                nc.vector.tensor_tensor(
                    out=a1tm[:, sl, :], in0=x_tm[:, sl, :],
                    in1=g1r[:, :, None].broadcast_to([P, nt, C]), op=OP.mult)
                nc.vector.tensor_tensor(
                    out=a1tm[:, sl, :], in0=a1tm[:, sl, :],
                    in1=mg1r[:, :, None].broadcast_to([P, nt, C]),
                    op=OP.subtract)
                for j in range(lo // 8, hi // 8):
                    pt = psT.tile([128, 4, 128], BF16, tag="tp")
                    for k in range(4):
                        tt = 8 * j + 2 * k
                        nc.tensor.transpose(out=pt[:, k, :],
                                            in_=a1tm_v[:, 64 * tt:64 * (tt + 2)],
                                            identity=ident)
                    nc.scalar.copy(out=a1cm_v[:, j, :, 0, :], in_=pt[0:C, :, :])
                    nc.vector.tensor_copy(out=a1cm_v[:, j, :, 1, :],
                                          in_=pt[C:128, :, :])
                    nc.sync.dma_start(
                        out=a1cm[C:128, 1024 * j:1024 * (j + 1) - 1],
                        in_=a1cm[0:C, 1024 * j + 1:1024 * (j + 1)])
                    if j > 0:
                        nc.gpsimd.tensor_copy(
                            out=a1cm[C:128, 1024 * j - 1:1024 * j],
                            in_=a1cm[0:C, 1024 * j:1024 * j + 1])

            # ---- spatial reduction conv (8x8 stride 8) ----
            a1sr = a1cm.rearrange("c (Y ky X kx) -> c ky kx Y X", ky=SR, kx=SR, X=16)
            psr = psS.tile([128, 512], F32, tag="pss", name="psr").rearrange(
                "c (a y x) -> c a y x", a=2, y=16)[0:C, 0, :, :]
            for pp in range(32):
                ky, kx = pp // 4, (pp % 4) * 2
                nc.tensor.matmul(out=psr, lhsT=wsr[:, pp, :],
                                 rhs=a1sr[:, ky, kx, :, :],
                                 start=(pp == 0), stop=(pp == 31))
            xrcm = consts.tile([C, NR], BF16)
            nc.scalar.activation(out=xrcm.rearrange("c (y x) -> c y x", x=16),
                                 in_=psr, func=AF.Identity,
                                 bias=bsr, scale=1.0)

            # ---- LN on reduced tokens (srn), token-major ----
            xr_tm = consts.tile([P, 2, C], F32)
            for hh in range(2):
                pv = psT.tile([128, C], BF16, tag="tp")
                nc.tensor.transpose(out=pv, in_=xrcm[:, 128 * hh:128 * (hh + 1)],
                                    identity=ident[0:C, 0:C])
                nc.vector.tensor_copy(out=xr_tm[:, hh, :], in_=pv)
            g_r, mg_r = _ln_stats(nc, sc, consts, xr_tm, epst, 2)
            ar_tm = consts.tile([P, 2, C], BF16)
            nc.vector.tensor_tensor(
                out=ar_tm, in0=xr_tm,
                in1=g_r[:, :, None].broadcast_to([P, 2, C]), op=OP.mult)
            mgb = sc.tile([P, 2, C], BF16, tag="mgb")
            nc.vector.tensor_tensor(
                out=mgb, in0=mg_r[:, :, None].broadcast_to([P, 2, C]),
                in1=g_r[:, :, None].broadcast_to([P, 2, C]), op=OP.bypass)
            nc.vector.tensor_tensor(out=ar_tm, in0=ar_tm, in1=mgb, op=OP.subtract)
            arcm = consts.tile([C, NR], BF16)
            for hh in range(2):
                pv = psT.tile([C, 128], BF16, tag="tp")
                nc.tensor.transpose(out=pv, in_=ar_tm[:, hh, :], identity=ident)
                nc.vector.tensor_copy(out=arcm[:, 128 * hh:128 * (hh + 1)], in_=pv)

            # ---- KV (V-half carries the folded proj weights) ----
            pkv = psS.tile([128, 512], F32, tag="pss", name="pkv")[:, 0:NR]
            nc.tensor.matmul(out=pkv, lhsT=wkv, rhs=arcm, start=True, stop=True)
            kvcm = consts.tile([2 * C, NR], BF16)
            nc.scalar.activation(out=kvcm, in_=pkv, func=AF.Identity,
                                 bias=bkv, scale=1.0)
            # fold q-projection into K:  S^T = (K @ Wq) @ A1
            bqb = consts.tile([C, 1], BF16)
            nc.vector.tensor_copy(out=bqb, in_=bq)
            pkw = psT.tile([C, NR], F32, tag="tp", name="pkw")
            nc.tensor.matmul(out=pkw, lhsT=wq, rhs=kvcm[0:C, :],
                             start=True, stop=True)
            # kwt: rows 0:64 = both kv halves; rows 64:128 = hh=1 half again
            # (so the row-tiled S^T pair can load its weights at base 64)
            kwt = consts.tile([128, NR], BF16)
            nc.scalar.copy(out=kwt[0:C, :], in_=pkw)
            nc.vector.tensor_copy(out=kwt[C:128, 0:128], in_=kwt[0:C, 128:256])
            sbias = consts.tile([128, 2], F32)
            for hh in range(2):
                pb = psT.tile([128, 1], F32, tag="tp", name="pb")
                nc.tensor.matmul(out=pb,
                                 lhsT=kvcm[0:C, 128 * hh:128 * (hh + 1)],
                                 rhs=bqb, start=True, stop=True)
                nc.vector.tensor_copy(out=sbias[:, hh:hh + 1], in_=pb)
            vp = consts.tile([128, 2, C + 1], BF16)
            nc.vector.memset(vp[:, :, C:C + 1], 1.0)
            for hh in range(2):
                pv = psT.tile([128, C], BF16, tag="tp")
                nc.tensor.transpose(out=pv,
                                    in_=kvcm[C:2 * C, 128 * hh:128 * (hh + 1)],
                                    identity=ident[C:2 * C, C:2 * C])
                nc.vector.tensor_copy(out=vp[:, hh, 0:C], in_=pv)
            # identity with an extra all-ones column: the proj transpose
            # then also emits per-token sums of o, giving the LN2 mean for
            # free (avoids a slow vector tensor_reduce per chunk)
            id66 = consts.tile([C + 1, C + 2], BF16)
            nc.vector.memset(id66[:, :], 0.0)
            nc.vector.tensor_copy(out=id66[:, 0:C + 1], in_=ident[0:C + 1, 0:C + 1])
            nc.vector.memset(id66[0:C, C + 1:C + 2], 1.0)

            # ---- attention (16 chunks of 1024) + a2 build + MLP + epi,
            #      emitted interleaved so all engines stay fed ----
            y_tm = big.tile([P, T, C], F32, tag="y")
            s1y = sc.tile([P, T], F32, tag="s1y")
            s2y = sc.tile([P, T], F32, tag="s2y")
            g2 = consts.tile([P, T], F32)
            mg2 = consts.tile([P, T], F32)
            a2tm = big.tile([P, T, C], BF16, tag="scr2", name="a2tm")
            a2tm_v = a2tm.rearrange("p t c -> p (t c)")
            a2g = big.tile([128, NG], BF16, tag="acm", name="a2g")
            a2rows = a2g[0:C, PAD + RP:PAD + RP * (H + 1)].rearrange(
                "c (y w) -> c y w", w=RP)
            a2rowsB = a2g[C:128, PAD + RP:PAD + RP * (H + 1)].rearrange(
                "c (y w) -> c y w", w=RP)
            ro = a2rows.rearrange("c (j a b) w -> c j a b w", a=4, b=2)
            o2cm = big.tile([128, NG], BF16, tag="qt")
            y2_tm = big.tile([P, T, C], F32, tag="xr", name="y2")
            out_v = out_d.rearrange("(p t) c -> p t c", t=T)

            ech_of = {}

            def emit_attn_S(i):
                """S^T (row-tiled K=64 pair) + exp for tokens
                [512*i, 512*(i+1))."""
                ech = ch.tile([128, 2, 512], BF16, tag="e")
                ech_of[i] = ech
                pS0 = psS.tile([128, 512], F32, tag="pss")
                nc.tensor.matmul(out=pS0, lhsT=kwt[0:C, 0:128],
                                 rhs=a1cm[0:C, 512 * i:512 * (i + 1)],
                                 start=True, stop=True)
                pS1 = psS.tile([128, 512], F32, tag="pss")
                if i == 0:
                    nc.tensor.matmul(out=pS1, lhsT=kwt[0:C, 128:256],
                                     rhs=a1cm[0:C, 0:512],
                                     start=True, stop=True)
                else:
                    nc.tensor.matmul(
                        out=pS1, lhsT=kwt[C:128, 0:128],
                        rhs=a1cm[C:128, 512 * i - 1:512 * (i + 1) - 1],
                        start=True, stop=True)
                for hh, pS in ((0, pS0), (1, pS1)):
                    nc.scalar.activation(out=ech[:, hh, :], in_=pS, func=AF.Exp,
                                         bias=sbias[:, hh:hh + 1], scale=1.0)

            def emit_attn_O(i):
                """Channel-major O (proj folded into V host-side), then
                token-major via identity matmuls; residual add + LN2 sums."""
                ech = ech_of.pop(i)
                pO = psS.tile([128, 512], F32, tag="pss", name="pO")[0:C + 1, :]
                for hh in range(2):
                    nc.tensor.matmul(out=pO, lhsT=vp[:, hh, :],
                                     rhs=ech[:, hh, :],
                                     start=(hh == 0), stop=(hh == 1))
                pod = ch.tile([C + 1, 512], BF16, tag="pod")
                if i % 2 == 0:
                    nc.vector.tensor_copy(out=pod, in_=pO)
                else:
                    nc.scalar.copy(out=pod, in_=pO)
                ptr = psP.tile([128, 4, C + 2], F32, tag="ptr")
                for k in range(4):
                    nc.tensor.matmul(out=ptr[:, k, :],
                                     lhsT=pod[:, 128 * k:128 * (k + 1)],
                                     rhs=id66,
                                     start=True, stop=True)
                rt = sc.tile([P, 4, 1], F32, tag="rt")
                nc.vector.reciprocal(out=rt, in_=ptr[:, :, C:C + 1])
                tmp = ch.tile([P, 4, C], F32, tag="tmp")
                nc.vector.tensor_tensor(out=tmp, in0=ptr[:, :, 0:C],
                                        in1=rt.broadcast_to([P, 4, C]),
                                        op=OP.mult)
                to = 4 * i
                nc.vector.tensor_tensor(out=y_tm[:, to:to + 4, :],
                                        in0=tmp, in1=x_tm[:, to:to + 4, :],
                                        op=OP.add)
                # LN2 running sums: sum(y) = sum(o)/denom + sum(x)
                pr = sc.tile([P, 4], F32, tag="pr")
                nc.vector.tensor_tensor(out=pr, in0=ptr[:, :, C + 1],
                                        in1=rt[:, :, 0], op=OP.mult)
                nc.vector.tensor_tensor(out=s1y[:, to:to + 4], in0=pr,
                                        in1=s1[:, to:to + 4], op=OP.add)
                if i % 2 == 1:
                    sl = slice(8 * (i // 2), 8 * (i // 2 + 1))
                    sq2 = sc.tile([P, 8, C], BF16, tag="sq8")
                    nc.vector.tensor_tensor(out=sq2, in0=y_tm[:, sl, :],
                                            in1=y_tm[:, sl, :], op=OP.mult)
                    tree_sum(s2y[:, sl], sq2, 8, C, False)

            def emit_a2_block(j):
                """LN2 scale + transpose + guarded store for image columns
                [8j, 8j+8) (g2/mg2 computed in one batch beforehand)."""
                sl = slice(8 * j, 8 * (j + 1))
                nc.vector.memset(a2rows[:, sl, 0:1], 0.0)
                nc.vector.memset(a2rows[:, sl, RP - 1:RP], 0.0)
                nc.vector.memset(a2rowsB[:, sl, RP - 2:RP], 0.0)
                nc.vector.tensor_tensor(
                    out=a2tm[:, sl, :], in0=y_tm[:, sl, :],
                    in1=g2[:, sl, None].broadcast_to([P, 8, C]), op=OP.mult)
                nc.vector.tensor_tensor(
                    out=a2tm[:, sl, :], in0=a2tm[:, sl, :],
                    in1=mg2[:, sl, None].broadcast_to([P, 8, C]),
                    op=OP.subtract)
                pt = psT.tile([128, 4, 128], BF16, tag="tp")
                for k in range(4):
                    tt = 8 * j + 2 * k
                    nc.tensor.transpose(out=pt[:, k, :],
                                        in_=a2tm_v[:, 64 * tt:64 * (tt + 2)],
                                        identity=ident)
                nc.scalar.copy(out=ro[:, j, :, 0, 1:W + 1], in_=pt[0:C, :, :])
                nc.vector.tensor_copy(out=ro[:, j, :, 1, 1:W + 1],
                                      in_=pt[C:128, :, :])
                nc.sync.dma_start(
                    out=a2rowsB[:, 8 * j:8 * (j + 1), 0:W],
                    in_=a2rows[:, 8 * j:8 * (j + 1), 1:W + 1])

            n_mlp = 33

            def emit_mlp_chunk(j):
                cb = PAD + RP + 512 * j
                size = min(512, PAD + RP * (H + 1) - cb)
                pG = [psS.tile([128, 512], F32, tag="pss", name=f"pg{g}")
                      for g in range(2)]
                for g in range(2):
                    for dy in (-1, 0, 1):
                        nc.tensor.matmul(
                            out=pG[g][:, 0:size], lhsT=wmp[:, 2 * (dy + 1) + g, :],
                            rhs=a2g[:, cb + RP * dy - 1:cb + RP * dy - 1 + size],
                            start=(dy == -1), stop=False)
                # row-tiled K=64 pairs: g0 on array rows 0:63 from the
                # direct rows, g1 on rows 64:127 from the shifted rows
                for dy in (-1, 0, 1):
                    nc.tensor.matmul(
                        out=pG[0][:, 0:size], lhsT=wms[0:C, dy + 1, :],
                        rhs=a2g[0:C, cb + RP * dy + 1:cb + RP * dy + 1 + size],
                        start=False, stop=(dy == 1))
                    nc.tensor.matmul(
                        out=pG[1][:, 0:size], lhsT=wms[C:128, dy + 1, :],
                        rhs=a2g[C:128, cb + RP * dy:cb + RP * dy + size],
                        start=False, stop=(dy == 1))
                gch = []
                for g in range(2):
                    gc = ch.tile([128, 512], BF16, tag=f"gc{g}")
                    nc.scalar.activation(out=gc[:, 0:size], in_=pG[g][:, 0:size],
                                         func=AF.Gelu, bias=bg[:, g:g + 1],
                                         scale=1.0)
                    gch.append(gc)
                pF = psS.tile([128, 512], F32, tag="pss", name="pf")
                for g in range(2):
                    nc.tensor.matmul(out=pF[0:C, 0:size], lhsT=wf2[:, g, :],
                                     rhs=gch[g][:, 0:size],
                                     start=(g == 0), stop=(g == 1))
                nc.vector.tensor_scalar(out=o2cm[0:C, cb:cb + size],
                                        in0=pF[0:C, 0:size], scalar1=bf2,
                                        scalar2=None, op0=OP.add)
                # doubled copy one row-pitch back so the epilogue transpose
                # reads two image rows per 128-partition window
                nc.sync.dma_start(out=o2cm[C:128, cb - RP:cb - RP + size],
                                  in_=o2cm[0:C, cb:cb + size])

            def emit_epi(e):
                """Transpose + residual-add for image columns 2e, 2e+1.
                The transpose rides the DMA xbar (square 128x128 bf16,
                SBUF->SBUF) instead of burning PE time in the PE-bound
                MLP phase."""
                s = PAD + RP * (2 * e + 1) + 1
                pt2 = psT.tile([128, 128], BF16, tag="tp", name="pt2")
                nc.tensor.transpose(out=pt2, in_=o2cm[:, s:s + 128],
                                    identity=ident)
                pt2v = pt2.rearrange("p (b c) -> p b c", c=C)
                nc.vector.tensor_tensor(out=y2_tm[:, 2 * e:2 * e + 2, :],
                                        in0=pt2v, in1=y_tm[:, 2 * e:2 * e + 2, :],
                                        op=OP.add)
                if e % 8 == 7:
                    q8 = e // 8
                    nc.sync.dma_start(out=out_v[:, 16 * q8:16 * (q8 + 1), :],
                                      in_=y2_tm[:, 16 * q8:16 * (q8 + 1), :])

            # tail pad lies beyond every a1cm column, safe to zero upfront
            nc.vector.memset(a2g[:, NG - PAD - RP:NG], 0.0)

            next_mlp = 0
            next_epi = 0
            done_a2 = 0

            def pump_mlp_epi(a2_blocks, mlp_budget):
                """Emit MLP chunks/epis whose inputs are complete."""
                nonlocal next_mlp, next_epi
                emitted = 0
                avail = RP * 8 * a2_blocks
                while (next_mlp < n_mlp and emitted < mlp_budget
                       and (a2_blocks >= 16
                            or 512 * (next_mlp + 1) + RP + 2 <= avail)):
                    emit_mlp_chunk(next_mlp)
                    next_mlp += 1
                    emitted += 1
                    while (next_epi < 64
                           and RP * (2 * next_epi + 1) + 129 <= 512 * next_mlp):
                        emit_epi(next_epi)
                        next_epi += 1

            # attention runs pure and software-pipelined: S^T(i+1) is
            # emitted before O(i) so the PE never waits on exp. a2 build +
            # MLP follow as a bridge — the lone SQRT and the gelus get
            # exactly one scalar table context each
            emit_attn_S(0)
            for i in range(1, 32):
                emit_attn_S(i)
                emit_attn_O(i - 1)
                # first half of the a2 build rides the late attention
                # chunks: stats for tokens < N/2 are complete by chunk 16,
                # and block j's a2g span stops aliasing a1cm once S^T
                # chunk 2j+2 has been emitted
                if i >= 16 and i % 2 == 0:
                    if i == 16:
                        nc.vector.memset(a2g[:, 0:PAD + RP], 0.0)
                        _ln_finalize(nc, sc, s1y[:, 0:64], s2y[:, 0:64],
                                     epst, 64, g2[:, 0:64], mg2[:, 0:64],
                                     tg="y")
                    emit_a2_block((i - 16) // 2)
            emit_attn_O(31)
            _ln_finalize(nc, sc, s1y[:, 64:128], s2y[:, 64:128], epst, 64,
                         g2[:, 64:128], mg2[:, 64:128], tg="y")
            # a2 blocks lead the MLP chunks by one block so the transpose->
            # copy->shift chain of block j overlaps the matmuls of block j-1
            for j in range(8, 16):
                emit_a2_block(j)
                pump_mlp_epi(j, 2)
            pump_mlp_epi(16, n_mlp)
            while next_epi < 64:
                emit_epi(next_epi)
                next_epi += 1

    _split_excess_waits(nc)
    return nc


@functools.cache
def _get_nc():
    return _build_nc()


def _prep_weights(inp):
    f = lambda v: np.asarray(v, np.float32)
    n1w, n1b = f(inp["n1_w"]), f(inp["n1_b"])
    q_w, q_b = f(inp["q_w"]), f(inp["q_b"])
    kv_w, kv_b = f(inp["kv_w"]), f(inp["kv_b"])
    sr_w, sr_b = f(inp["sr_w"]), f(inp["sr_b"])
    srnw, srnb = f(inp["srn_w"]), f(inp["srn_b"])
    pj_w, pj_b = f(inp["proj_w"]), f(inp["proj_b"])
    n2w, n2b = f(inp["n2_w"]), f(inp["n2_b"])
    f1w, f1b = f(inp["fc1_w"]), f(inp["fc1_b"])
    dww, dwb = f(inp["dw_w"]), f(inp["dw_b"])
    f2w, f2b = f(inp["fc2_w"]), f(inp["fc2_b"])

    scale = (C // 1) ** -0.5
    wq_l = (q_w * n1w[None, :]).T * scale
    bq_l = ((q_w @ n1b + q_b) * scale)[:, None]

    # image arrives transposed (see x_v layout): swap the two spatial
    # tap indices of both convs
    wsr_l = np.zeros((32, 128, C), np.float32)
    for pp in range(32):
        ky, kx = pp // 4, (pp % 4) * 2
        wsr_l[pp, :C, :] = (sr_w[:, :, kx, ky] * n1w[None, :]).T
        wsr_l[pp, C:, :] = (sr_w[:, :, kx + 1, ky] * n1w[None, :]).T
    wsr_l = wsr_l.transpose(1, 0, 2)
    bsr_l = (sr_w.sum((2, 3)) @ n1b + sr_b)[:, None]

    # fold the output projection into the V half of the KV linear:
    # Vp = V @ pj_w.T + pj_b, and the denominator divides out pj_b's
    # contribution via the fused ones-column
    kvp_w = np.concatenate([kv_w[:C], pj_w @ kv_w[C:]], 0)
    kvp_b = np.concatenate([kv_b[:C], pj_w @ kv_b[C:] + pj_b], 0)
    wkv_l = (kvp_w * srnw[None, :]).T
    bkv_l = (kvp_w @ srnb + kvp_b)[:, None]

    k9 = dww[:, 0, :, :].transpose(0, 2, 1).reshape(HID, 9)  # taps transposed
    wmp_l = np.zeros((6, 128, 128), np.float32)
    wms_l = np.zeros((3, 128, 128), np.float32)
    for dy in range(3):
        for g in range(2):
            Ma = (k9[:, dy * 3 + 0][:, None] * f1w * n2w[None, :])[128 * g:128 * (g + 1)]
            Mb = (k9[:, dy * 3 + 1][:, None] * f1w * n2w[None, :])[128 * g:128 * (g + 1)]
            Mc = (k9[:, dy * 3 + 2][:, None] * f1w * n2w[None, :])[128 * g:128 * (g + 1)]
            wmp_l[2 * dy + g, :C, :] = Ma.T
            wmp_l[2 * dy + g, C:, :] = Mb.T
            wms_l[dy, 64 * g:64 * (g + 1), :] = Mc.T
    wmp_l = wmp_l.transpose(1, 0, 2)
    wms_l = wms_l.transpose(1, 0, 2)
    bg_full = k9.sum(1) * (f1w @ n2b + f1b) + dwb  # [256]
    bg_l = np.ascontiguousarray(bg_full.reshape(2, 128).T)

    wf2_l = np.stack([f2w[:, :128].T, f2w[:, 128:].T], 0).transpose(1, 0, 2)
    bf2_l = f2b[:, None]

    bfc = lambda a: np.ascontiguousarray(a).astype(BF)
    return {
        "wq": bfc(wq_l), "bq": np.ascontiguousarray(bq_l),
        "wsr": bfc(wsr_l), "bsr": np.ascontiguousarray(bsr_l),
        "wkv": bfc(wkv_l), "bkv": np.ascontiguousarray(bkv_l),
        "wmp": bfc(wmp_l), "wms": bfc(wms_l),
        "bg": np.ascontiguousarray(bg_l),
        "wf2": bfc(wf2_l), "bf2": np.ascontiguousarray(bf2_l),
    }


def kernel(trace=False, tmpdir=None, **inputs):
    nc = _get_nc()
    x = np.asarray(inputs["x"], np.float32)
    wts = _prep_weights(inputs)
    in_maps = [dict(wts, x=np.ascontiguousarray(x[b])) for b in range(B)]
    res = run_bass_kernel_spmd(nc, in_maps, core_ids=list(range(8)),
                               trace=trace, tmpdir=tmpdir)
    out = np.stack([res.results[b]["out"] for b in range(B)], 0)
    kernel.last_exec_time_ns = res.exec_time_ns
    return out
